# revision 33
# baseline (speedup 1.0000x reference)
"""Trainium2 Bass kernel for nn_AudioMixer (4-track stereo mixer:
per-track 3-stage biquad EQ -> compressor -> Schroeder reverb on tracks 2,3
-> pan/volume mix -> limiter clip).

Sharding: core c = (track c//2, channel c%2) — each of the 8 cores processes
one full (track, channel) row of 1.44M samples end-to-end, then a
ReduceScatter over channel groups {0,2,4,6} / {1,3,5,7} sums the 4 weighted
tracks per channel; each core clips + writes a quarter of its channel.

Key optimizations over the f32 baseline (810us -> 560us cost-model time):
 - All EQ/transpose matmuls run in float32r (1 cycle/row vs 4 for f32;
   requires even moving dims, producers declared f32r end-to-end).
 - EQ cross-block corrections use an exact state-space scheme: s_in = F@x
   per 128-block (one matmul), corrections = stacked (Phi A^i) matmul over
   shifted s_in windows. Well-conditioned (|entries| <= 46 vs 543 for the
   old probe-fit tails scheme, whose 5e-2 error under f32r rounding was the
   accuracy blocker); no serial y-tail chain; 5x less stack DMA.
 - Compressor (policy-iteration envelope) keeps true-f32 state (f32r's
   1.2e-4 ulp stalls the env recurrence: 1-REL ~ 2e-4), 4 iterations,
   scan op1=subtract sign trick folds coef prep into one stt, gain via
   Relu/Ln/Exp on Act with the clamp folded into the Ln affine.
 - Reverb epoch-filter matmuls in bf16; inputs gather-cast f32->bf16 via
   SWDGE so strided reads move half the bytes (bf16 DRAM scratch crashes
   the runtime, so buffers stay f32 and only reads are cast).
 - Two-half software pipeline: the signal splits into flat halves with
   per-half 128-lane compressor layouts (env chain crosses halves via a
   scalar seed), so reverb DMA/PE of half 0 overlaps compressor DVE of
   half 1; reverb tile grids split per stage at receding cutoffs.
"""
import math
from contextlib import ExitStack

import numpy as np

import concourse.bass as bass
import concourse.bacc as bacc
import concourse.mybir as mybir
import concourse.tile as tile
from concourse.bass_utils import run_bass_kernel_spmd

F32 = mybir.dt.float32
F32R = mybir.dt.float32r
BF16 = mybir.dt.bfloat16

# ---------------------------------------------------------------- constants
SR = 48000
N = 1_440_000
NP = 128 * 11264          # padded row length (1441792)
F = 11264                 # per-lane length (128 lanes)
FCB = 2816                # EQ free-chunk (4 chunks)
FC = 1408                 # compressor free-chunk (8 chunks)
CH = NP // 4              # ReduceScatter chunk per core = 360448

ATK = math.exp(-1.0 / (10.0 * 0.001 * SR))
REL = math.exp(-1.0 / (100.0 * 0.001 * SR))
THR = 10.0 ** (-18.0 / 20.0)
GR_EXP = 1.0 / 4.0 - 1.0
_BASE = int(SR * 0.03)
COMB_DELAYS = [_BASE, int(_BASE * 1.13), int(_BASE * 1.27), int(_BASE * 1.41)]
AP_DELAYS = [int(SR * 0.005), int(SR * 0.0017)]
FB = 0.3 + 0.5 * 0.6
WET = 0.3
CEIL = 10.0 ** (-1.0 / 20.0)

N_ITER = 4                # compressor policy iterations (lazy chaining)
NST = 6                   # EQ state dimension (3 biquads)
JM = 8                    # correction shift terms (block-decay truncation)
VEP = 104               # valid epochs per reverb tile
WEP = 24                # warmup epochs per tile (fb^24 ~ 4.6e-5)

# ---------------------------------------------------------------- EQ host math
def _peak_coefs(freq, gain_db, q):
    A = 10.0 ** (gain_db / 40.0)
    w0 = 2.0 * math.pi * freq / SR
    al = math.sin(w0) / (2.0 * q)
    a0 = 1.0 + al / A
    return ((1.0 + al * A) / a0, -2.0 * math.cos(w0) / a0, (1.0 - al * A) / a0,
            -2.0 * math.cos(w0) / a0, (1.0 - al / A) / a0)

_IDENT = (1.0, 0.0, 0.0, 0.0, 0.0)
_PRESETS = {
    0: [(300.0, -3.0, 0.7), (3000.0, 3.0, 1.0), (8000.0, 2.0, 0.7)],
    1: [(80.0, 2.0, 0.7), (5000.0, 1.0, 1.0)],
    2: [(200.0, -2.0, 0.7), (6000.0, -1.0, 0.7)],
    3: [(1000.0, 2.0, 1.0)],
}

def _stage_coefs(track):
    bands = [_peak_coefs(*b) for b in _PRESETS[track]]
    bands += [_IDENT] * (3 - len(bands))
    return bands

def _biquad_ss(c):
    b0, b1, b2, a1, a2 = [float(v) for v in c]
    A = np.array([[-a1, 1.0], [-a2, 0.0]])
    B = np.array([[b1 - a1 * b0], [b2 - a2 * b0]])
    C = np.array([[1.0, 0.0]])
    D = np.array([[b0]])
    return A, B, C, D

def _cascade(ss_list):
    A1, B1, C1, D1 = ss_list[0]
    for A2, B2, C2, D2 in ss_list[1:]:
        n1, n2 = A1.shape[0], A2.shape[0]
        A = np.zeros((n1 + n2, n1 + n2))
        A[:n1, :n1] = A1
        A[n1:, :n1] = B2 @ C1
        A[n1:, n1:] = A2
        B = np.vstack([B1, B2 @ D1])
        C = np.hstack([D2 @ C1, C2])
        D = D2 @ D1
        A1, B1, C1, D1 = A, B, C, D
    return A1, B1, C1, D1

def _track_eq_consts(track, L=128):
    A, B, C, D = _cascade([_biquad_ss(c) for c in _stage_coefs(track)])
    n = A.shape[0]          # 6 == NST
    h = np.zeros(L)
    h[0] = D[0, 0]
    Ak = np.eye(n)
    for k in range(1, L):
        h[k] = (C @ Ak @ B)[0, 0]
        Ak = A @ Ak
    T = np.zeros((L, L))
    for i in range(L):
        T[i, : i + 1] = h[i::-1]
    Phi = np.zeros((L, n))
    Ak = np.eye(n)
    for k in range(L):
        Phi[k] = (C @ Ak)[0]
        Ak = A @ Ak
    A_L = Ak
    # F[:, j] = A^(L-1-j) B: state contribution of one block of inputs
    Fm = np.zeros((n, L))
    Ak = np.eye(n)
    for j in range(L - 1, -1, -1):
        Fm[:, j] = (Ak @ B)[:, 0]
        Ak = A @ Ak
    # stacked correction lhsT: row (i', c) = (Phi A_L^i)[:, c], i = JM-1-i'
    SC = np.zeros((n * JM, L))
    Ai = np.eye(n)
    for i in range(JM):
        PA = Phi @ Ai
        ip = JM - 1 - i
        for c in range(n):
            SC[n * ip + c] = PA[:, c]
        Ai = A_L @ Ai
    return T, Fm, SC

# ---------------------------------------------------------------- reverb host math
def _epoch_matrix_comb(fb, L=128):
    Lm = np.zeros((L, L))
    for q in range(L):
        y = np.zeros(L)
        prev = 0.0
        for p_ in range(L):
            y[p_] = (1.0 if p_ == q else 0.0) + fb * prev
            prev = y[p_]
        Lm[:, q] = y
    return Lm

def _epoch_matrix_ap(fb, L=128, quirk=False):
    Lm = np.zeros((L, L))
    for q in range(L):
        X = np.zeros(L)
        X[q] = 1.0
        y = np.zeros(L)
        yprev = 0.0
        xprev = 0.0
        for p_ in range(L):
            y[p_] = 0.0 if (quirk and p_ == 0) else (-fb * X[p_] + xprev + fb * yprev)
            yprev = y[p_]
            xprev = X[p_]
        Lm[:, q] = y
    return Lm

def _rev_tiles(d):
    M = -(-NP // d)
    T = -(-M // VEP)
    return T, T * VEP * d   # tile count, flat coverage

_COMB_COVER = max(_rev_tiles(d)[1] for d in COMB_DELAYS)
_AP0_COVER = _rev_tiles(AP_DELAYS[0])[1]
_AP1_COVER = _rev_tiles(AP_DELAYS[1])[1]
# ap81 reads apdram up to its own tile-grid coverage; ap240 only writes its
# grid's coverage -> size apdram to the max and zero the gap
_AP0_SIZE = max(_AP0_COVER, _AP1_COVER)


# ============================================================== device program
def build_program(with_collective=True, phase_limit=4,
                  r_trans=True, r_eq=True, r_corr=True, r_rev=True):
    nc = bacc.Bacc("TRN2", target_bir_lowering=False, debug=False)
    dt = F32R
    c_tr = (lambda a: a) if r_trans else (lambda a: a.bitcast(F32))
    c_eq = (lambda a: a) if r_eq else (lambda a: a.bitcast(F32))
    c_co = (lambda a: a) if r_corr else (lambda a: a.bitcast(F32))
    c_rv = (lambda a: a) if r_rev else (lambda a: a.bitcast(F32))
    ao = mybir.AluOpType
    AF = mybir.ActivationFunctionType

    x = nc.declare_dram_parameter("x", [NP], dt, isOutput=False)
    thT = nc.declare_dram_parameter("thT", [128, 128], dt, isOutput=False)
    ftp = nc.declare_dram_parameter("ft", [128, NST], dt, isOutput=False)
    scp = nc.declare_dram_parameter("sc", [NST * JM, 128], dt, isOutput=False)
    identp = nc.declare_dram_parameter("ident", [128, 128], dt, isOutput=False)
    lcT = nc.declare_dram_parameter("lcT", [128, 128], dt, isOutput=False)
    laT = nc.declare_dram_parameter("laT", [128, 128], dt, isOutput=False)
    laTw = nc.declare_dram_parameter("laTw", [128, 128], dt, isOutput=False)
    laqTw = nc.declare_dram_parameter("laqTw", [128, 128], dt, isOutput=False)
    laqT = nc.declare_dram_parameter("laqT", [128, 128], dt, isOutput=False)
    relpow = nc.declare_dram_parameter("relpow", [128, FC], F32, isOutput=False)
    wdry = nc.declare_dram_parameter("wdry", [128, 1], F32, isOutput=False)
    wwet = nc.declare_dram_parameter("wwet", [128, 1], F32, isOutput=False)
    out = nc.declare_dram_parameter("out", [CH], F32, isOutput=True)

    ydram = nc.dram_tensor("ydram", [_COMB_COVER], dt)
    wetdram = nc.dram_tensor("wetdram", [_COMB_COVER], dt)
    apdram = nc.dram_tensor("apdram", [_AP0_SIZE], dt)
    wet2dram = nc.dram_tensor("wet2dram", [_AP1_COVER], dt)
    mixdram = nc.dram_tensor("mixdram", [max(NP, _AP1_COVER)], dt)
    sindram = nc.dram_tensor("sindram", [NST, 32 + F], dt)
    rsdram = nc.dram_tensor("rsdram", [CH], dt)

    with tile.TileContext(nc) as tc, ExitStack() as ctx:
        cons = ctx.enter_context(tc.tile_pool(name="cons", bufs=1))
        bigs = ctx.enter_context(tc.tile_pool(name="bigs", bufs=3))
        ps = ctx.enter_context(tc.tile_pool(name="ps", bufs=4, space="PSUM"))
        tiny = ctx.enter_context(tc.tile_pool(name="tiny", bufs=2))

        # ---- constants to SBUF
        t_thT = cons.tile([128, 128], dt, tag="thT")
        t_ft = cons.tile([128, NST], dt, tag="ft")
        t_sc = cons.tile([NST * JM, 128], dt, tag="sc")
        t_id = cons.tile([128, 128], dt, tag="ident")
        t_lcT = cons.tile([128, 128], dt, tag="lcT")
        t_laT = cons.tile([128, 128], dt, tag="laT")
        t_laTw = cons.tile([128, 128], dt, tag="laTw")
        t_laqTw = cons.tile([128, 128], dt, tag="laqTw")
        t_laqT = cons.tile([128, 128], dt, tag="laqT")
        t_relpow = cons.tile([128, FC], F32, tag="relpow")
        t_wdry = cons.tile([128, 1], F32, tag="wdry")
        t_wwet = cons.tile([128, 1], F32, tag="wwet")
        t_ones = cons.tile([1, 1], dt, tag="ones")
        t_zcol = cons.tile([128, 1], F32, tag="zcol")
        for t_, src in ((t_thT, thT), (t_ft, ftp), (t_sc, scp), (t_id, identp),
                        (t_lcT, lcT), (t_laT, laT), (t_laqT, laqT),
                        (t_laTw, laTw), (t_laqTw, laqTw),
                        (t_relpow, relpow), (t_wdry, wdry), (t_wwet, wwet)):
            nc.sync.dma_start(t_[:], src[:])
        nc.gpsimd.memset(t_ones[:], 1.0)
        nc.gpsimd.memset(t_zcol[:], 0.0)

        # ================= Phase A: load x -> U tiles -> PE transpose -> xL1
        xL1 = bigs.tile([128, F], dt, tag="big")
        x4 = x[:].rearrange("(w a b) -> w a b", a=128, b=128)  # [88,128,128]
        with tc.tile_pool(name="stg", bufs=4) as stg:
            for wq8 in range(11):
                s = stg.tile([128, 8, 128], dt, tag="ustg")
                nc.sync.dma_start(
                    s[:], x4[8 * wq8: 8 * wq8 + 8].rearrange("w a b -> a w b"))
                for half in range(2):
                    wq = 2 * wq8 + half
                    pt = ps.tile([128, 512], dt, tag="pstrans")
                    for wl in range(4):
                        nc.tensor.transpose(
                            c_tr(pt[:, 128 * wl: 128 * wl + 128]),
                            c_tr(s[:, 4 * half + wl, :]), c_tr(t_id[:]))
                    nc.scalar.copy(xL1[:, 512 * wq: 512 * wq + 512], pt[:])

        # ================= Phase B: EQ matmuls (exact state-space corr)
        ytr = bigs.tile([128, F], dt, tag="big")
        SUBS = [512] * 5 + [256]   # 2816
        with tc.tile_pool(name="stk", bufs=2) as stkp:
            zpad = stkp.tile([NST, 32], dt, tag="zpad")
            nc.gpsimd.memset(zpad[:].bitcast(F32), 0.0)
            nc.sync.dma_start(sindram[:, 0:32], zpad[:])
            for k in range(4):
                base = FCB * k
                # s_in for this chunk -> sindram
                sinc = stkp.tile([NST, FCB], dt, tag="sinc")
                off = 0
                for sub in SUBS:
                    p1 = ps.tile([128, 512], F32, tag="psmm")
                    nc.tensor.matmul(p1[:NST, :sub], c_eq(t_ft[:]),
                                     c_eq(xL1[:, base + off: base + off + sub]))
                    nc.scalar.copy(sinc[:, off: off + sub], p1[:NST, :sub])
                    off += sub
                nc.sync.dma_start(sindram[:, 32 + base: 32 + base + FCB],
                                  sinc[:])
                # stacked shifted s_in for corrections
                sct = stkp.tile([NST * JM, FCB], dt, tag="stack")
                soff = 32 + base - JM
                sap = [[1, JM], [32 + F, NST], [1, FCB]]
                nc.sync.dma_start(
                    sct[:], bass.AP(tensor=sindram, offset=soff, ap=sap))
                off = 0
                for sub in SUBS:
                    p1 = ps.tile([128, 512], F32, tag="psmm")
                    nc.tensor.matmul(p1[:, :sub], c_eq(t_thT[:]),
                                     c_eq(xL1[:, base + off: base + off + sub]),
                                     start=True, stop=False)
                    nc.tensor.matmul(p1[:, :sub], c_co(t_sc[:]),
                                     c_co(sct[:, off: off + sub]),
                                     start=False, stop=True)
                    nc.scalar.copy(ytr[:, base + off: base + off + sub],
                                   p1[:, :sub])
                    off += sub

        # ================= Phase C: transpose back + flatten to L2 lanes
        ustg2 = bigs.tile([128, F], dt, tag="big")   # reuses xL1's slot region
        for wq in range(22):
            pt = ps.tile([128, 512], dt, tag="pstrans")
            for wl in range(4):
                w = 4 * wq + wl
                nc.tensor.transpose(c_tr(pt[:, 128 * wl: 128 * wl + 128]),
                                    c_tr(ytr[:, 128 * w: 128 * w + 128]),
                                    c_tr(t_id[:]))
            nc.scalar.copy(ustg2[:, 512 * wq: 512 * wq + 512], pt[:])
        yeq = bigs.tile([128, F], dt, tag="big")     # reuses y0's slot region
        # flatten via DRAM scratch (mixdram is free here): tile-major -> flat
        u3 = ustg2[:].rearrange("a (w b) -> a w b", b=128)
        md = mixdram[0:NP].rearrange("(w a b) -> a w b", a=128, b=128)
        u3g = ustg2[:].rearrange("a (w b) -> a w b", b=128)
        for kt in range(4):
            nc.sync.dma_start(md[:, 22 * kt: 22 * (kt + 1), :],
                              u3g[:, 22 * kt: 22 * (kt + 1), :])

        # ========== Phase D/E/F: two-half software pipeline ==========
        # Half h covers flat [h*HNP, (h+1)*HNP); its lane-major layout is
        # lane p = flat[h*HNP + p*FH : +FH], held in yeq/lvl/env columns
        # [h*FH, (h+1)*FH). Compressor(h1) overlaps reverb(h0): the env
        # chain crosses halves via a scalar seed (h0 final env).
        FH = F // 2
        HNP = NP // 2
        KC = 4

        lvl = bigs.tile([128, F], F32, tag="big")    # reuses ytr's slot
        env = bigs.tile([128, F], F32, tag="big")    # reuses ustg2's slot
        mdvh = [mixdram[h * HNP: (h + 1) * HNP].rearrange("(p f) -> p f",
                                                          p=128)
                for h in (0, 1)]
        ydvh = [ydram[h * HNP: (h + 1) * HNP].rearrange("(p f) -> p f", p=128)
                for h in (0, 1)]
        w2vh = [wet2dram[h * HNP: (h + 1) * HNP].rearrange("(p f) -> p f",
                                                           p=128)
                for h in (0, 1)]
        sc_thr = float(1.0 / (THR + 1e-8))

        with tc.tile_pool(name="relcp", bufs=1) as relcp, \
             tc.tile_pool(name="chk", bufs=2) as chk, \
             tc.tile_pool(name="rvin", bufs=4) as rvin, \
             tc.tile_pool(name="rvout", bufs=2) as rvout:
            relc = relcp.tile([128, FC], F32, tag="relc")
            nc.gpsimd.memset(relc[:].bitcast(F32), REL)
            rowsc = tiny.tile([1, 130], F32, tag="rowsc")
            irow = tiny.tile([1, 128], F32, tag="irow")
            nc.gpsimd.memset(rowsc[:].bitcast(F32), 0.0)
            relFrow = tiny.tile([1, 128], F32, tag="relF")
            nc.gpsimd.memset(relFrow[:].bitcast(F32), float(REL ** FH))
            icol = tiny.tile([128, 1], F32, tag="icol")
            nthr = tiny.tile([128, 1], F32, tag="nthr")
            nc.gpsimd.memset(nthr[:].bitcast(F32), float(-(THR + 1e-8)))
            lanecol = tiny.tile([128, 1], F32, tag="lanecol")
            savec = tiny.tile([128, 8], F32, tag="savec")

            # zero tail of ydram (combs read past NP)
            zt = rvout.tile([128, 2048], dt, tag="rv_out")
            nc.gpsimd.memset(zt[:].bitcast(F32), 0.0)
            tail = _COMB_COVER - NP
            tf = tail // 2048
            nc.sync.dma_start(
                ydram[NP: NP + tf * 2048].rearrange("(o f) -> o f", o=tf),
                zt[0:tf, :])
            rem = tail - tf * 2048
            if rem:
                nc.sync.dma_start(
                    ydram[NP + tf * 2048:].rearrange("(o f) -> o f", o=1),
                    zt[tf: tf + 1, 0:rem])

            def gen_compress(h):
                o = FH * h
                for k in range(KC):
                    c0 = o + FC * k
                    nc.sync.dma_start(yeq[:, c0: c0 + FC],
                                      mdvh[h][:, FC * k: FC * (k + 1)])
                    nc.scalar.activation(lvl[:, c0: c0 + FC],
                                         yeq[:, c0: c0 + FC], AF.Abs)
                yield
                # it0: all-release + exact chain fix
                for k in range(KC):
                    c0 = o + FC * k
                    d1 = chk.tile([128, FC], F32, tag="d1")
                    nc.scalar.mul(d1[:], lvl[:, c0: c0 + FC], 1.0 - REL)
                    init = (0.0 if h == 0 else rowsc[0:1, 129:130]) \
                        if k == 0 else env[:, c0 - 1: c0]
                    nc.vector.tensor_tensor_scan(
                        env[:, c0: c0 + FC], relc[:], d1[:],
                        0.0 if k == 0 else init, op0=ao.mult, op1=ao.add)
                yield
                pr = ps.tile([128, 512], F32, tag="psmm")
                nc.tensor.matmul(pr[:1, :128],
                                 env[:, o + FH - 1: o + FH].bitcast(F32),
                                 t_id[:].bitcast(F32))
                nc.scalar.copy(rowsc[0:1, 1:129], pr[:1, :128])
                if h == 0:
                    nc.vector.tensor_copy(rowsc[0:1, 0:1], t_zcol[0:1, :])
                else:
                    # seed: h0 final (fixed) env, from partition 127 via DMA
                    nc.sync.dma_start(rowsc[0:1, 0:1],
                                      env[127:128, FH - 1: FH])
                nc.vector.tensor_tensor_scan(
                    irow[:], relFrow[:], rowsc[0:1, 0:128], 0.0,
                    op0=ao.mult, op1=ao.add)
                pc = ps.tile([128, 512], F32, tag="psmm")
                nc.tensor.matmul(pc[:128, :1], irow[:].bitcast(F32),
                                 t_ones[:].bitcast(F32))
                nc.scalar.copy(icol[:], pc[:128, :1])
                for k in range(KC):
                    c0 = o + FC * k
                    isc = tiny.tile([128, 1], F32, tag="isc")
                    nc.vector.tensor_scalar_mul(isc[:], icol[:],
                                                float(REL ** (FC * k)))
                    nc.vector.scalar_tensor_tensor(
                        env[:, c0: c0 + FC], t_relpow[:], isc[:, 0:1],
                        env[:, c0: c0 + FC], op0=ao.mult, op1=ao.add)
                yield
                for it in range(N_ITER):
                    last = it == N_ITER - 1
                    nc.sync.dma_start(lanecol[1:128, :],
                                      env[0:127, o + FH - 1: o + FH])
                    if h == 0:
                        nc.vector.tensor_copy(lanecol[0:1, :], t_zcol[0:1, :])
                    else:
                        nc.sync.dma_start(lanecol[0:1, :],
                                          env[127:128, FH - 1: FH])
                    for k in range(KC):
                        nc.vector.tensor_copy(
                            savec[:, k: k + 1],
                            env[:, o + FC * (k + 1) - 1: o + FC * (k + 1)])
                    mts, ncts = [], []
                    for k in range(KC):
                        c0 = o + FC * k
                        m = chk.tile([128, FC], F32, tag="mtile")
                        bc = lanecol[:, 0:1] if k == 0 else savec[:, k - 1: k]
                        nc.vector.tensor_tensor(
                            m[:, 1:], lvl[:, c0 + 1: c0 + FC],
                            env[:, c0: c0 + FC - 1], op=ao.is_gt)
                        nc.vector.tensor_tensor(
                            m[:, 0:1], lvl[:, c0: c0 + 1], bc, op=ao.is_gt)
                        nc.scalar.activation(m[:], m[:], AF.Copy,
                                             bias=float(REL),
                                             scale=float(ATK - REL))
                        nct = chk.tile([128, FC], F32, tag="d1")
                        nc.vector.scalar_tensor_tensor(
                            nct[:], m[:], 1.0, lvl[:, c0: c0 + FC],
                            op0=ao.subtract, op1=ao.mult)
                        mts.append(m)
                        ncts.append(nct)
                    yield
                    for k in range(KC):
                        c0 = o + FC * k
                        init = lanecol[:, 0:1] if k == 0 else \
                            env[:, c0 - 1: c0]
                        nc.vector.tensor_tensor_scan(
                            env[:, c0: c0 + FC], mts[k][:], ncts[k][:], init,
                            op0=ao.mult, op1=ao.subtract)
                        if last:
                            # gain + ydram write for this chunk immediately
                            a = chk.tile([128, FC], F32, tag="mtile")
                            b = chk.tile([128, FC], F32, tag="d1")
                            nc.scalar.activation(b[:], env[:, c0: c0 + FC],
                                                 AF.Relu, bias=nthr[:, 0:1])
                            nc.scalar.activation(a[:], b[:], AF.Ln,
                                                 scale=sc_thr, bias=1.0)
                            nc.scalar.activation(b[:], a[:], AF.Exp,
                                                 scale=float(GR_EXP))
                            nc.vector.tensor_mul(yeq[:, c0: c0 + FC],
                                                 yeq[:, c0: c0 + FC], b[:])
                            nc.sync.dma_start(
                                ydvh[h][:, FC * k: FC * (k + 1)],
                                yeq[:, c0: c0 + FC])
                    yield
                # dry gain (after ydram writes consumed yeq)
                nc.scalar.activation(yeq[:, o: o + FH], yeq[:, o: o + FH],
                                     AF.Copy, scale=t_wdry[:, 0:1])

            def epoch_filter(src_dram, dst_dram, d, lhsT_t0, lhsT, accum,
                             G=1, in_dt=dt, t_lo=0, t_hi=None):
                in_eng = nc.gpsimd if in_dt != src_dram.dtype else nc.sync
                Tt, cover = _rev_tiles(d)
                if t_hi is None:
                    t_hi = Tt
                packmm = max(1, 512 // d)

                def do_group(t, g):
                    it_ = rvin.tile([128, g, d], in_dt, tag="rv_in")
                    if t == 0:
                        in_eng.dma_start(
                            it_[:, 0, :],
                            src_dram[0: 128 * d].rearrange("(e i) -> e i",
                                                           e=128))
                    else:
                        soff = (VEP * t - WEP) * d
                        in_eng.dma_start(
                            it_[:], bass.AP(tensor=src_dram, offset=soff,
                                            ap=[[d, 128], [VEP * d, g],
                                                [1, d]]))
                    ot = rvout.tile([128, g, d], dt, tag="rv_out")
                    lt = lhsT_t0 if t == 0 else lhsT
                    j = 0
                    while j < g:
                        pk = min(packmm, g - j)
                        sub = pk * d
                        ev = nc.scalar.copy if (t % 2 == 0) else \
                            nc.vector.tensor_copy
                        if pk == 1:
                            offd = 0
                            while offd < d:
                                s2 = min(512, d - offd)
                                pe = ps.tile([128, 512], F32, tag="psmm")
                                rhs = it_[:, j, offd: offd + s2]
                                if lt.tensor.dtype != BF16 and s2 % 2:
                                    nc.tensor.matmul(pe[:, :s2],
                                                     lt[:].bitcast(F32),
                                                     rhs.bitcast(F32))
                                else:
                                    nc.tensor.matmul(pe[:, :s2], lt[:], rhs)
                                ev(ot[:, j, offd: offd + s2], pe[:, :s2])
                                offd += s2
                        else:
                            pe = ps.tile([128, 512], F32, tag="psmm")
                            pv = pe[:, :sub].rearrange("p (j i) -> p j i",
                                                       j=pk)
                            rhs = it_[:, j: j + pk, :]
                            if lt.tensor.dtype != BF16 and sub % 2:
                                nc.tensor.matmul(pv, lt[:].bitcast(F32),
                                                 rhs.bitcast(F32))
                            else:
                                nc.tensor.matmul(pv, lt[:], rhs)
                            ev(ot[:, j: j + pk, :], pv)
                        j += pk
                    if t == 0:
                        rows = ot[0:VEP, 0, :]
                        dst = dst_dram[0: VEP * d].rearrange("(e i) -> e i",
                                                             e=VEP)
                        eng = nc.gpsimd if accum else nc.sync
                        eng.dma_start(dst, rows,
                                      **({"accum_op": ao.add} if accum
                                         else {}))
                    else:
                        if not accum:
                            dap = bass.AP(tensor=dst_dram,
                                          offset=VEP * t * d,
                                          ap=[[d, VEP], [VEP * d, g],
                                              [1, d]])
                            nc.sync.dma_start(dap, ot[WEP:128, :, :])
                        else:
                            j0 = 0
                            while j0 < g:
                                gg = min(8, g - j0)
                                dap = bass.AP(
                                    tensor=dst_dram,
                                    offset=(VEP * (t + j0)) * d,
                                    ap=[[d, VEP], [VEP * d, gg], [1, d]])
                                nc.gpsimd.dma_start(
                                    dap, ot[WEP:128, j0: j0 + gg, :],
                                    accum_op=ao.add)
                                j0 += gg

                if t_lo == 0:
                    do_group(0, 1)
                    yield
                t = max(1, t_lo)
                while t < t_hi:
                    g = min(G, t_hi - t)
                    do_group(t, g)
                    t += g
                    yield

            # per-stage h0/h1 tile cutoffs (each stage only reads what the
            # previous one has produced in its h0 pass)
            combs = sorted(COMB_DELAYS, key=lambda d: -_rev_tiles(d)[1])
            T0c = {d: HNP // (VEP * d) for d in combs}
            C1 = min(VEP * d * T0c[d] for d in combs)
            T0a0 = C1 // (VEP * AP_DELAYS[0])
            C2 = VEP * AP_DELAYS[0] * T0a0
            T0a1 = C2 // (VEP * AP_DELAYS[1])
            C3 = VEP * AP_DELAYS[1] * T0a1
            L0 = min((C3 // FH) // 32 * 32, 128)

            def drain(*gens, weights=None):
                gens = list(gens)
                ws = list(weights) if weights else [1] * len(gens)
                while gens:
                    nxt, nw = [], []
                    for g, w in zip(gens, ws):
                        alive = True
                        for _ in range(w):
                            try:
                                next(g)
                            except StopIteration:
                                alive = False
                                break
                        if alive:
                            nxt.append(g)
                            nw.append(w)
                    gens, ws = nxt, nw

            def gen_chain(*gens):
                for g in gens:
                    yield from g

            def gen_mix(h, p0, p1):
                for k in range(KC):
                    c0 = FH * h + FC * k
                    wetb = rvin.tile([128, FC], BF16, tag="rv_in")
                    mixt = chk.tile([128, FC], F32, tag="mtile")
                    nc.gpsimd.dma_start(wetb[p0:p1, :],
                                        w2vh[h][p0:p1, FC * k: FC * (k + 1)])
                    nc.vector.scalar_tensor_tensor(
                        mixt[p0:p1, :], wetb[p0:p1, :], t_wwet[p0:p1, 0:1],
                        yeq[p0:p1, c0: c0 + FC], op0=ao.mult, op1=ao.add)
                    nc.sync.dma_start(mdvh[h][p0:p1, FC * k: FC * (k + 1)],
                                      mixt[p0:p1, :].bitcast(F32R))
                    yield

            def gen_gap_zero():
                if _AP0_SIZE > _AP0_COVER:
                    gap = _AP0_SIZE - _AP0_COVER
                    ztg = rvout.tile([128, 2048], dt, tag="rv_out")
                    nc.gpsimd.memset(ztg[:].bitcast(F32), 0.0)
                    nc.sync.dma_start(
                        apdram[_AP0_COVER:].rearrange("(o f) -> o f", o=1),
                        ztg[0:1, 0:gap])
                yield

            # ---------------- pipeline ----------------
            drain(gen_compress(0))
            rev_all = gen_chain(
                *[epoch_filter(ydram, wetdram, d, t_lcT, t_lcT,
                               accum=(ci > 0), in_dt=BF16, t_hi=T0c[d])
                  for ci, d in enumerate(combs)],
                epoch_filter(wetdram, apdram, AP_DELAYS[0], t_laqT, t_laT,
                             False, G=8, in_dt=BF16, t_hi=T0a0),
                gen_gap_zero(),
                epoch_filter(apdram, wet2dram, AP_DELAYS[1], t_laqT, t_laT,
                             False, G=24, in_dt=BF16, t_hi=T0a1),
                gen_mix(0, 0, L0),
                *[epoch_filter(ydram, wetdram, d, t_lcT, t_lcT,
                               accum=(ci > 0), in_dt=BF16, t_lo=T0c[d])
                  for ci, d in enumerate(combs)],
                epoch_filter(wetdram, apdram, AP_DELAYS[0], t_laqT, t_laT,
                             False, G=8, in_dt=BF16, t_lo=T0a0),
                epoch_filter(apdram, wet2dram, AP_DELAYS[1], t_laqT, t_laT,
                             False, G=24, in_dt=BF16, t_lo=T0a1),
                gen_mix(0, L0, 128),
                gen_mix(1, 0, 128))
            drain(rev_all, gen_compress(1), weights=(1, 3))

            if with_collective:
                nc.gpsimd.collective_compute(
                    "ReduceScatter", ao.add,
                    replica_groups=[[0, 2, 4, 6], [1, 3, 5, 7]],
                    ins=[mixdram[0:NP].opt()],
                    outs=[rsdram.ap().opt()],
                )
            else:
                nc.sync.dma_start(rsdram[:], mixdram[0:CH])
            rs2 = rsdram[:].rearrange("(p f) -> p f", p=128)
            o2 = out[:].rearrange("(p f) -> p f", p=128)
            for hh in range(2):
                oc = rvout.tile([128, FC], dt, tag="rv_out")
                nc.sync.dma_start(oc[:], rs2[:, FC * hh: FC * (hh + 1)])
                nc.vector.tensor_scalar(oc[:], oc[:], float(-CEIL),
                                        float(CEIL), op0=ao.max, op1=ao.min)
                nc.sync.dma_start(o2[:, FC * hh: FC * (hh + 1)],
                                  oc[:].bitcast(F32))

    nc.compile()
    return nc


# ============================================================== host wrapper
_CACHE = {}

def _get_program():
    if "nc" not in _CACHE:
        _CACHE["nc"] = build_program()
    return _CACHE["nc"]


def _host_consts():
    if "consts" in _CACHE:
        return _CACHE["consts"]
    ident = np.eye(128, dtype=np.float32)
    Lc = np.ascontiguousarray((0.25 * _epoch_matrix_comb(FB)).T.astype(np.float32))
    La = np.ascontiguousarray(_epoch_matrix_ap(FB).T.astype(np.float32))
    Laq = np.ascontiguousarray(_epoch_matrix_ap(FB, quirk=True).T.astype(np.float32))
    relpow = np.ascontiguousarray(np.broadcast_to(
        (REL ** (np.arange(FC, dtype=np.float64) + 1.0)).astype(np.float32),
        (128, FC)))
    eqc = {}
    for t in range(4):
        T, Fm, SC = _track_eq_consts(t)
        eqc[t] = (np.ascontiguousarray(T.T.astype(np.float32)),
                  np.ascontiguousarray(Fm.T.astype(np.float32)),
                  np.ascontiguousarray(SC.astype(np.float32)))
    _CACHE["consts"] = (ident, Lc, La, Laq, relpow, eqc)
    return _CACHE["consts"]


def kernel(tracks, volumes, pans):
    tracks = np.ascontiguousarray(np.asarray(tracks, np.float32))
    volumes = np.asarray(volumes, np.float32)
    pans = np.asarray(pans, np.float32)

    angle = (pans.astype(np.float64) + 1.0) * 0.25 * math.pi
    lg, rg = np.cos(angle), np.sin(angle)
    ident, Lc, La, Laq, relpow, eqc = _host_consts()

    in_maps = []
    for core in range(8):
        t, ch = core // 2, core % 2
        xpad = np.zeros(NP, np.float32)
        xpad[:N] = tracks[t, ch]
        thT_np, ft_np, sc_np = eqc[t]
        w = float(volumes[t]) * float(lg[t] if ch == 0 else rg[t])
        has_rev = t >= 2
        w_dry = w * (1.0 - WET) if has_rev else w
        w_wet = w * WET if has_rev else 0.0
        in_maps.append({
            "x": xpad, "thT": thT_np, "ft": ft_np, "sc": sc_np,
            "ident": ident, "lcT": Lc, "laT": La, "laqT": Laq,
            "laTw": np.ascontiguousarray(La * np.float32(w_wet)),
            "laqTw": np.ascontiguousarray(Laq * np.float32(w_wet)),
            "relpow": relpow,
            "wdry": np.full((128, 1), w_dry, np.float32),
            "wwet": np.full((128, 1), w_wet, np.float32),
        })

    nc = _get_program()
    res = run_bass_kernel_spmd(nc, in_maps, list(range(8)))

    outp = np.zeros((2, N), np.float32)
    for ch in range(2):
        full = np.concatenate([res.results[2 * q + ch]["out"] for q in range(4)])
        outp[ch] = full[:N]
    return outp



# revision 34
# speedup vs baseline: 1.0035x; 1.0035x over previous
"""Trainium2 Bass kernel for nn_AudioMixer (4-track stereo mixer:
per-track 3-stage biquad EQ -> compressor -> Schroeder reverb on tracks 2,3
-> pan/volume mix -> limiter clip).

Sharding: core c = (track c//2, channel c%2) — each of the 8 cores processes
one full (track, channel) row of 1.44M samples end-to-end, then a
ReduceScatter over channel groups {0,2,4,6} / {1,3,5,7} sums the 4 weighted
tracks per channel; each core clips + writes a quarter of its channel.

Key optimizations over the f32 baseline (810us -> 560us cost-model time):
 - All EQ/transpose matmuls run in float32r (1 cycle/row vs 4 for f32;
   requires even moving dims, producers declared f32r end-to-end).
 - EQ cross-block corrections use an exact state-space scheme: s_in = F@x
   per 128-block (one matmul), corrections = stacked (Phi A^i) matmul over
   shifted s_in windows. Well-conditioned (|entries| <= 46 vs 543 for the
   old probe-fit tails scheme, whose 5e-2 error under f32r rounding was the
   accuracy blocker); no serial y-tail chain; 5x less stack DMA.
 - Compressor (policy-iteration envelope) keeps true-f32 state (f32r's
   1.2e-4 ulp stalls the env recurrence: 1-REL ~ 2e-4), 4 iterations,
   scan op1=subtract sign trick folds coef prep into one stt, gain via
   Relu/Ln/Exp on Act with the clamp folded into the Ln affine.
 - Reverb epoch-filter matmuls in bf16; inputs gather-cast f32->bf16 via
   SWDGE so strided reads move half the bytes (bf16 DRAM scratch crashes
   the runtime, so buffers stay f32 and only reads are cast).
 - Two-half software pipeline: the signal splits into flat halves with
   per-half 128-lane compressor layouts (env chain crosses halves via a
   scalar seed), so reverb DMA/PE of half 0 overlaps compressor DVE of
   half 1; reverb tile grids split per stage at receding cutoffs.
"""
import math
from contextlib import ExitStack

import numpy as np

import concourse.bass as bass
import concourse.bacc as bacc
import concourse.mybir as mybir
import concourse.tile as tile
from concourse.bass_utils import run_bass_kernel_spmd

F32 = mybir.dt.float32
F32R = mybir.dt.float32r
BF16 = mybir.dt.bfloat16

# ---------------------------------------------------------------- constants
SR = 48000
N = 1_440_000
NP = 128 * 11264          # padded row length (1441792)
F = 11264                 # per-lane length (128 lanes)
FCB = 2816                # EQ free-chunk (4 chunks)
FC = 1408                 # compressor free-chunk (8 chunks)
CH = NP // 4              # ReduceScatter chunk per core = 360448

ATK = math.exp(-1.0 / (10.0 * 0.001 * SR))
REL = math.exp(-1.0 / (100.0 * 0.001 * SR))
THR = 10.0 ** (-18.0 / 20.0)
GR_EXP = 1.0 / 4.0 - 1.0
_BASE = int(SR * 0.03)
COMB_DELAYS = [_BASE, int(_BASE * 1.13), int(_BASE * 1.27), int(_BASE * 1.41)]
AP_DELAYS = [int(SR * 0.005), int(SR * 0.0017)]
FB = 0.3 + 0.5 * 0.6
WET = 0.3
CEIL = 10.0 ** (-1.0 / 20.0)

N_ITER = 4                # compressor policy iterations (lazy chaining)
NST = 6                   # EQ state dimension (3 biquads)
JM = 8                    # correction shift terms (block-decay truncation)
VEP = 104               # valid epochs per reverb tile
WEP = 24                # warmup epochs per tile (fb^24 ~ 4.6e-5)

# ---------------------------------------------------------------- EQ host math
def _peak_coefs(freq, gain_db, q):
    A = 10.0 ** (gain_db / 40.0)
    w0 = 2.0 * math.pi * freq / SR
    al = math.sin(w0) / (2.0 * q)
    a0 = 1.0 + al / A
    return ((1.0 + al * A) / a0, -2.0 * math.cos(w0) / a0, (1.0 - al * A) / a0,
            -2.0 * math.cos(w0) / a0, (1.0 - al / A) / a0)

_IDENT = (1.0, 0.0, 0.0, 0.0, 0.0)
_PRESETS = {
    0: [(300.0, -3.0, 0.7), (3000.0, 3.0, 1.0), (8000.0, 2.0, 0.7)],
    1: [(80.0, 2.0, 0.7), (5000.0, 1.0, 1.0)],
    2: [(200.0, -2.0, 0.7), (6000.0, -1.0, 0.7)],
    3: [(1000.0, 2.0, 1.0)],
}

def _stage_coefs(track):
    bands = [_peak_coefs(*b) for b in _PRESETS[track]]
    bands += [_IDENT] * (3 - len(bands))
    return bands

def _biquad_ss(c):
    b0, b1, b2, a1, a2 = [float(v) for v in c]
    A = np.array([[-a1, 1.0], [-a2, 0.0]])
    B = np.array([[b1 - a1 * b0], [b2 - a2 * b0]])
    C = np.array([[1.0, 0.0]])
    D = np.array([[b0]])
    return A, B, C, D

def _cascade(ss_list):
    A1, B1, C1, D1 = ss_list[0]
    for A2, B2, C2, D2 in ss_list[1:]:
        n1, n2 = A1.shape[0], A2.shape[0]
        A = np.zeros((n1 + n2, n1 + n2))
        A[:n1, :n1] = A1
        A[n1:, :n1] = B2 @ C1
        A[n1:, n1:] = A2
        B = np.vstack([B1, B2 @ D1])
        C = np.hstack([D2 @ C1, C2])
        D = D2 @ D1
        A1, B1, C1, D1 = A, B, C, D
    return A1, B1, C1, D1

def _track_eq_consts(track, L=128):
    A, B, C, D = _cascade([_biquad_ss(c) for c in _stage_coefs(track)])
    n = A.shape[0]          # 6 == NST
    h = np.zeros(L)
    h[0] = D[0, 0]
    Ak = np.eye(n)
    for k in range(1, L):
        h[k] = (C @ Ak @ B)[0, 0]
        Ak = A @ Ak
    T = np.zeros((L, L))
    for i in range(L):
        T[i, : i + 1] = h[i::-1]
    Phi = np.zeros((L, n))
    Ak = np.eye(n)
    for k in range(L):
        Phi[k] = (C @ Ak)[0]
        Ak = A @ Ak
    A_L = Ak
    # F[:, j] = A^(L-1-j) B: state contribution of one block of inputs
    Fm = np.zeros((n, L))
    Ak = np.eye(n)
    for j in range(L - 1, -1, -1):
        Fm[:, j] = (Ak @ B)[:, 0]
        Ak = A @ Ak
    # stacked correction lhsT: row (i', c) = (Phi A_L^i)[:, c], i = JM-1-i'
    SC = np.zeros((n * JM, L))
    Ai = np.eye(n)
    for i in range(JM):
        PA = Phi @ Ai
        ip = JM - 1 - i
        for c in range(n):
            SC[n * ip + c] = PA[:, c]
        Ai = A_L @ Ai
    return T, Fm, SC

# ---------------------------------------------------------------- reverb host math
def _epoch_matrix_comb(fb, L=128):
    Lm = np.zeros((L, L))
    for q in range(L):
        y = np.zeros(L)
        prev = 0.0
        for p_ in range(L):
            y[p_] = (1.0 if p_ == q else 0.0) + fb * prev
            prev = y[p_]
        Lm[:, q] = y
    return Lm

def _epoch_matrix_ap(fb, L=128, quirk=False):
    Lm = np.zeros((L, L))
    for q in range(L):
        X = np.zeros(L)
        X[q] = 1.0
        y = np.zeros(L)
        yprev = 0.0
        xprev = 0.0
        for p_ in range(L):
            y[p_] = 0.0 if (quirk and p_ == 0) else (-fb * X[p_] + xprev + fb * yprev)
            yprev = y[p_]
            xprev = X[p_]
        Lm[:, q] = y
    return Lm

def _rev_tiles(d):
    M = -(-NP // d)
    T = -(-M // VEP)
    return T, T * VEP * d   # tile count, flat coverage

_COMB_COVER = max(_rev_tiles(d)[1] for d in COMB_DELAYS)
_AP0_COVER = _rev_tiles(AP_DELAYS[0])[1]
_AP1_COVER = _rev_tiles(AP_DELAYS[1])[1]
# ap81 reads apdram up to its own tile-grid coverage; ap240 only writes its
# grid's coverage -> size apdram to the max and zero the gap
_AP0_SIZE = max(_AP0_COVER, _AP1_COVER)


# ============================================================== device program
def build_program(with_collective=True, phase_limit=4,
                  r_trans=True, r_eq=True, r_corr=True, r_rev=True):
    nc = bacc.Bacc("TRN2", target_bir_lowering=False, debug=False)
    dt = F32R
    c_tr = (lambda a: a) if r_trans else (lambda a: a.bitcast(F32))
    c_eq = (lambda a: a) if r_eq else (lambda a: a.bitcast(F32))
    c_co = (lambda a: a) if r_corr else (lambda a: a.bitcast(F32))
    c_rv = (lambda a: a) if r_rev else (lambda a: a.bitcast(F32))
    ao = mybir.AluOpType
    AF = mybir.ActivationFunctionType

    x = nc.declare_dram_parameter("x", [NP], dt, isOutput=False)
    thT = nc.declare_dram_parameter("thT", [128, 128], dt, isOutput=False)
    ftp = nc.declare_dram_parameter("ft", [128, NST], dt, isOutput=False)
    scp = nc.declare_dram_parameter("sc", [NST * JM, 128], dt, isOutput=False)
    identp = nc.declare_dram_parameter("ident", [128, 128], dt, isOutput=False)
    lcT = nc.declare_dram_parameter("lcT", [128, 128], dt, isOutput=False)
    laT = nc.declare_dram_parameter("laT", [128, 128], dt, isOutput=False)
    laTw = nc.declare_dram_parameter("laTw", [128, 128], dt, isOutput=False)
    laqTw = nc.declare_dram_parameter("laqTw", [128, 128], dt, isOutput=False)
    laqT = nc.declare_dram_parameter("laqT", [128, 128], dt, isOutput=False)
    relpow = nc.declare_dram_parameter("relpow", [128, FC], F32, isOutput=False)
    wdry = nc.declare_dram_parameter("wdry", [128, 1], F32, isOutput=False)
    wwet = nc.declare_dram_parameter("wwet", [128, 1], F32, isOutput=False)
    out = nc.declare_dram_parameter("out", [CH], F32, isOutput=True)

    ydram = nc.dram_tensor("ydram", [_COMB_COVER], dt)
    wetdram = nc.dram_tensor("wetdram", [_COMB_COVER], dt)
    apdram = nc.dram_tensor("apdram", [_AP0_SIZE], dt)
    wet2dram = nc.dram_tensor("wet2dram", [_AP1_COVER], dt)
    mixdram = nc.dram_tensor("mixdram", [max(NP, _AP1_COVER)], dt)
    sindram = nc.dram_tensor("sindram", [NST, 32 + F], dt)
    rsdram = nc.dram_tensor("rsdram", [CH], dt)

    with tile.TileContext(nc) as tc, ExitStack() as ctx:
        cons = ctx.enter_context(tc.tile_pool(name="cons", bufs=1))
        bigs = ctx.enter_context(tc.tile_pool(name="bigs", bufs=3))
        ps = ctx.enter_context(tc.tile_pool(name="ps", bufs=4, space="PSUM"))
        tiny = ctx.enter_context(tc.tile_pool(name="tiny", bufs=2))

        # ---- constants to SBUF
        t_thT = cons.tile([128, 128], dt, tag="thT")
        t_ft = cons.tile([128, NST], dt, tag="ft")
        t_sc = cons.tile([NST * JM, 128], dt, tag="sc")
        t_id = cons.tile([128, 128], dt, tag="ident")
        t_lcT = cons.tile([128, 128], dt, tag="lcT")
        t_laT = cons.tile([128, 128], dt, tag="laT")
        t_laTw = cons.tile([128, 128], dt, tag="laTw")
        t_laqTw = cons.tile([128, 128], dt, tag="laqTw")
        t_laqT = cons.tile([128, 128], dt, tag="laqT")
        t_relpow = cons.tile([128, FC], F32, tag="relpow")
        t_wdry = cons.tile([128, 1], F32, tag="wdry")
        t_wwet = cons.tile([128, 1], F32, tag="wwet")
        t_ones = cons.tile([1, 1], dt, tag="ones")
        t_zcol = cons.tile([128, 1], F32, tag="zcol")
        for t_, src in ((t_thT, thT), (t_ft, ftp), (t_sc, scp), (t_id, identp),
                        (t_lcT, lcT), (t_laT, laT), (t_laqT, laqT),
                        (t_laTw, laTw), (t_laqTw, laqTw),
                        (t_relpow, relpow), (t_wdry, wdry), (t_wwet, wwet)):
            nc.sync.dma_start(t_[:], src[:])
        nc.gpsimd.memset(t_ones[:], 1.0)
        nc.gpsimd.memset(t_zcol[:], 0.0)

        # ================= Phase A: load x -> U tiles -> PE transpose -> xL1
        xL1 = bigs.tile([128, F], dt, tag="big")
        x4 = x[:].rearrange("(w a b) -> w a b", a=128, b=128)  # [88,128,128]
        with tc.tile_pool(name="stg", bufs=4) as stg:
            for wq8 in range(11):
                s = stg.tile([128, 8, 128], dt, tag="ustg")
                nc.sync.dma_start(
                    s[:], x4[8 * wq8: 8 * wq8 + 8].rearrange("w a b -> a w b"))
                for half in range(2):
                    wq = 2 * wq8 + half
                    pt = ps.tile([128, 512], dt, tag="pstrans")
                    for wl in range(4):
                        nc.tensor.transpose(
                            c_tr(pt[:, 128 * wl: 128 * wl + 128]),
                            c_tr(s[:, 4 * half + wl, :]), c_tr(t_id[:]))
                    nc.scalar.copy(xL1[:, 512 * wq: 512 * wq + 512], pt[:])

        # ================= Phase B: EQ matmuls (exact state-space corr)
        ytr = bigs.tile([128, F], dt, tag="big")
        SUBS = [512] * 5 + [256]   # 2816
        with tc.tile_pool(name="stk", bufs=2) as stkp:
            zpad = stkp.tile([NST, 32], dt, tag="zpad")
            nc.gpsimd.memset(zpad[:].bitcast(F32), 0.0)
            nc.sync.dma_start(sindram[:, 0:32], zpad[:])
            for k in range(4):
                base = FCB * k
                # s_in for this chunk -> sindram
                sinc = stkp.tile([NST, FCB], dt, tag="sinc")
                off = 0
                for sub in SUBS:
                    p1 = ps.tile([128, 512], F32, tag="psmm")
                    nc.tensor.matmul(p1[:NST, :sub], c_eq(t_ft[:]),
                                     c_eq(xL1[:, base + off: base + off + sub]))
                    nc.scalar.copy(sinc[:, off: off + sub], p1[:NST, :sub])
                    off += sub
                nc.sync.dma_start(sindram[:, 32 + base: 32 + base + FCB],
                                  sinc[:])
                # stacked shifted s_in for corrections
                sct = stkp.tile([NST * JM, FCB], dt, tag="stack")
                soff = 32 + base - JM
                sap = [[1, JM], [32 + F, NST], [1, FCB]]
                nc.sync.dma_start(
                    sct[:], bass.AP(tensor=sindram, offset=soff, ap=sap))
                off = 0
                for sub in SUBS:
                    p1 = ps.tile([128, 512], F32, tag="psmm")
                    nc.tensor.matmul(p1[:, :sub], c_eq(t_thT[:]),
                                     c_eq(xL1[:, base + off: base + off + sub]),
                                     start=True, stop=False)
                    nc.tensor.matmul(p1[:, :sub], c_co(t_sc[:]),
                                     c_co(sct[:, off: off + sub]),
                                     start=False, stop=True)
                    nc.scalar.copy(ytr[:, base + off: base + off + sub],
                                   p1[:, :sub])
                    off += sub

        # ================= Phase C: transpose back + flatten to L2 lanes
        ustg2 = bigs.tile([128, F], dt, tag="big")   # reuses xL1's slot region
        for wq in range(22):
            pt = ps.tile([128, 512], dt, tag="pstrans")
            for wl in range(4):
                w = 4 * wq + wl
                nc.tensor.transpose(c_tr(pt[:, 128 * wl: 128 * wl + 128]),
                                    c_tr(ytr[:, 128 * w: 128 * w + 128]),
                                    c_tr(t_id[:]))
            nc.scalar.copy(ustg2[:, 512 * wq: 512 * wq + 512], pt[:])
        yeq = bigs.tile([128, F], dt, tag="big")     # reuses y0's slot region
        # flatten via DRAM scratch (mixdram is free here): tile-major -> flat
        u3 = ustg2[:].rearrange("a (w b) -> a w b", b=128)
        md = mixdram[0:NP].rearrange("(w a b) -> a w b", a=128, b=128)
        u3g = ustg2[:].rearrange("a (w b) -> a w b", b=128)
        for kt in range(4):
            nc.sync.dma_start(md[:, 22 * kt: 22 * (kt + 1), :],
                              u3g[:, 22 * kt: 22 * (kt + 1), :])

        # ========== Phase D/E/F: two-half software pipeline ==========
        # Half h covers flat [h*HNP, (h+1)*HNP); its lane-major layout is
        # lane p = flat[h*HNP + p*FH : +FH], held in yeq/lvl/env columns
        # [h*FH, (h+1)*FH). Compressor(h1) overlaps reverb(h0): the env
        # chain crosses halves via a scalar seed (h0 final env).
        FH = F // 2
        HNP = NP // 2
        KC = 4

        lvl = bigs.tile([128, F], F32, tag="big")    # reuses ytr's slot
        env = bigs.tile([128, F], F32, tag="big")    # reuses ustg2's slot
        mdvh = [mixdram[h * HNP: (h + 1) * HNP].rearrange("(p f) -> p f",
                                                          p=128)
                for h in (0, 1)]
        ydvh = [ydram[h * HNP: (h + 1) * HNP].rearrange("(p f) -> p f", p=128)
                for h in (0, 1)]
        w2vh = [wet2dram[h * HNP: (h + 1) * HNP].rearrange("(p f) -> p f",
                                                           p=128)
                for h in (0, 1)]
        sc_thr = float(1.0 / (THR + 1e-8))

        with tc.tile_pool(name="relcp", bufs=1) as relcp, \
             tc.tile_pool(name="chk", bufs=2) as chk, \
             tc.tile_pool(name="rvin", bufs=4) as rvin, \
             tc.tile_pool(name="rvout", bufs=2) as rvout:
            relc = relcp.tile([128, FC], F32, tag="relc")
            nc.gpsimd.memset(relc[:].bitcast(F32), REL)
            rowsc = tiny.tile([1, 130], F32, tag="rowsc")
            irow = tiny.tile([1, 128], F32, tag="irow")
            nc.gpsimd.memset(rowsc[:].bitcast(F32), 0.0)
            relFrow = tiny.tile([1, 128], F32, tag="relF")
            nc.gpsimd.memset(relFrow[:].bitcast(F32), float(REL ** FH))
            icol = tiny.tile([128, 1], F32, tag="icol")
            nthr = tiny.tile([128, 1], F32, tag="nthr")
            nc.gpsimd.memset(nthr[:].bitcast(F32), float(-(THR + 1e-8)))
            lanecol = tiny.tile([128, 1], F32, tag="lanecol")
            savec = tiny.tile([128, 8], F32, tag="savec")

            # zero tail of ydram (combs read past NP)
            zt = rvout.tile([128, 2048], dt, tag="rv_out")
            nc.gpsimd.memset(zt[:].bitcast(F32), 0.0)
            tail = _COMB_COVER - NP
            tf = tail // 2048
            nc.sync.dma_start(
                ydram[NP: NP + tf * 2048].rearrange("(o f) -> o f", o=tf),
                zt[0:tf, :])
            rem = tail - tf * 2048
            if rem:
                nc.sync.dma_start(
                    ydram[NP + tf * 2048:].rearrange("(o f) -> o f", o=1),
                    zt[tf: tf + 1, 0:rem])

            def gen_compress(h):
                o = FH * h
                for k in range(KC):
                    c0 = o + FC * k
                    nc.sync.dma_start(yeq[:, c0: c0 + FC],
                                      mdvh[h][:, FC * k: FC * (k + 1)])
                    nc.scalar.activation(lvl[:, c0: c0 + FC],
                                         yeq[:, c0: c0 + FC], AF.Abs)
                yield
                # it0: all-release + exact chain fix
                for k in range(KC):
                    c0 = o + FC * k
                    d1 = chk.tile([128, FC], F32, tag="d1")
                    nc.scalar.mul(d1[:], lvl[:, c0: c0 + FC], 1.0 - REL)
                    init = (0.0 if h == 0 else rowsc[0:1, 129:130]) \
                        if k == 0 else env[:, c0 - 1: c0]
                    nc.vector.tensor_tensor_scan(
                        env[:, c0: c0 + FC], relc[:], d1[:],
                        0.0 if k == 0 else init, op0=ao.mult, op1=ao.add)
                yield
                pr = ps.tile([128, 512], F32, tag="psmm")
                nc.tensor.matmul(pr[:1, :128],
                                 env[:, o + FH - 1: o + FH].bitcast(F32),
                                 t_id[:].bitcast(F32))
                nc.scalar.copy(rowsc[0:1, 1:129], pr[:1, :128])
                if h == 0:
                    nc.vector.tensor_copy(rowsc[0:1, 0:1], t_zcol[0:1, :])
                else:
                    # seed: h0 final (fixed) env, from partition 127 via DMA
                    nc.sync.dma_start(rowsc[0:1, 0:1],
                                      env[127:128, FH - 1: FH])
                nc.vector.tensor_tensor_scan(
                    irow[:], relFrow[:], rowsc[0:1, 0:128], 0.0,
                    op0=ao.mult, op1=ao.add)
                pc = ps.tile([128, 512], F32, tag="psmm")
                nc.tensor.matmul(pc[:128, :1], irow[:].bitcast(F32),
                                 t_ones[:].bitcast(F32))
                nc.scalar.copy(icol[:], pc[:128, :1])
                for k in range(KC):
                    c0 = o + FC * k
                    isc = tiny.tile([128, 1], F32, tag="isc")
                    nc.vector.tensor_scalar_mul(isc[:], icol[:],
                                                float(REL ** (FC * k)))
                    nc.vector.scalar_tensor_tensor(
                        env[:, c0: c0 + FC], t_relpow[:], isc[:, 0:1],
                        env[:, c0: c0 + FC], op0=ao.mult, op1=ao.add)
                yield
                for it in range(N_ITER):
                    last = it == N_ITER - 1
                    nc.sync.dma_start(lanecol[1:128, :],
                                      env[0:127, o + FH - 1: o + FH])
                    if h == 0:
                        nc.vector.tensor_copy(lanecol[0:1, :], t_zcol[0:1, :])
                    else:
                        nc.sync.dma_start(lanecol[0:1, :],
                                          env[127:128, FH - 1: FH])
                    for k in range(KC):
                        nc.vector.tensor_copy(
                            savec[:, k: k + 1],
                            env[:, o + FC * (k + 1) - 1: o + FC * (k + 1)])
                    mts, ncts = [], []
                    for k in range(KC):
                        c0 = o + FC * k
                        m = chk.tile([128, FC], F32, tag="mtile")
                        bc = lanecol[:, 0:1] if k == 0 else savec[:, k - 1: k]
                        nc.vector.tensor_tensor(
                            m[:, 1:], lvl[:, c0 + 1: c0 + FC],
                            env[:, c0: c0 + FC - 1], op=ao.is_gt)
                        nc.vector.tensor_tensor(
                            m[:, 0:1], lvl[:, c0: c0 + 1], bc, op=ao.is_gt)
                        nc.scalar.activation(m[:], m[:], AF.Copy,
                                             bias=float(REL),
                                             scale=float(ATK - REL))
                        nct = chk.tile([128, FC], F32, tag="d1")
                        nc.vector.scalar_tensor_tensor(
                            nct[:], m[:], 1.0, lvl[:, c0: c0 + FC],
                            op0=ao.subtract, op1=ao.mult)
                        mts.append(m)
                        ncts.append(nct)
                    yield
                    for k in range(KC):
                        c0 = o + FC * k
                        init = lanecol[:, 0:1] if k == 0 else \
                            env[:, c0 - 1: c0]
                        nc.vector.tensor_tensor_scan(
                            env[:, c0: c0 + FC], mts[k][:], ncts[k][:], init,
                            op0=ao.mult, op1=ao.subtract)
                        if last:
                            # gain + ydram write for this chunk immediately
                            a = chk.tile([128, FC], F32, tag="mtile")
                            b = chk.tile([128, FC], F32, tag="d1")
                            nc.scalar.activation(b[:], env[:, c0: c0 + FC],
                                                 AF.Relu, bias=nthr[:, 0:1])
                            nc.scalar.activation(a[:], b[:], AF.Ln,
                                                 scale=sc_thr, bias=1.0)
                            nc.scalar.activation(b[:], a[:], AF.Exp,
                                                 scale=float(GR_EXP))
                            nc.vector.tensor_mul(yeq[:, c0: c0 + FC],
                                                 yeq[:, c0: c0 + FC], b[:])
                            nc.sync.dma_start(
                                ydvh[h][:, FC * k: FC * (k + 1)],
                                yeq[:, c0: c0 + FC])
                    yield
                # dry gain (after ydram writes consumed yeq)
                nc.scalar.activation(yeq[:, o: o + FH], yeq[:, o: o + FH],
                                     AF.Copy, scale=t_wdry[:, 0:1])

            def epoch_filter(src_dram, dst_dram, d, lhsT_t0, lhsT, accum,
                             G=1, in_dt=dt, t_lo=0, t_hi=None):
                in_eng = nc.gpsimd if in_dt != src_dram.dtype else nc.sync
                Tt, cover = _rev_tiles(d)
                if t_hi is None:
                    t_hi = Tt
                packmm = max(1, 512 // d)

                def do_group(t, g):
                    it_ = rvin.tile([128, g, d], in_dt, tag="rv_in")
                    if t == 0:
                        in_eng.dma_start(
                            it_[:, 0, :],
                            src_dram[0: 128 * d].rearrange("(e i) -> e i",
                                                           e=128))
                    else:
                        soff = (VEP * t - WEP) * d
                        in_eng.dma_start(
                            it_[:], bass.AP(tensor=src_dram, offset=soff,
                                            ap=[[d, 128], [VEP * d, g],
                                                [1, d]]))
                    ot = rvout.tile([128, g, d], dt, tag="rv_out")
                    lt = lhsT_t0 if t == 0 else lhsT
                    j = 0
                    while j < g:
                        pk = min(packmm, g - j)
                        sub = pk * d
                        ev = nc.scalar.copy if (t % 2 == 0) else \
                            nc.vector.tensor_copy
                        if pk == 1:
                            offd = 0
                            while offd < d:
                                s2 = min(512, d - offd)
                                pe = ps.tile([128, 512], F32, tag="psmm")
                                rhs = it_[:, j, offd: offd + s2]
                                if lt.tensor.dtype != BF16 and s2 % 2:
                                    nc.tensor.matmul(pe[:, :s2],
                                                     lt[:].bitcast(F32),
                                                     rhs.bitcast(F32))
                                else:
                                    nc.tensor.matmul(pe[:, :s2], lt[:], rhs)
                                ev(ot[:, j, offd: offd + s2], pe[:, :s2])
                                offd += s2
                        else:
                            pe = ps.tile([128, 512], F32, tag="psmm")
                            pv = pe[:, :sub].rearrange("p (j i) -> p j i",
                                                       j=pk)
                            rhs = it_[:, j: j + pk, :]
                            if lt.tensor.dtype != BF16 and sub % 2:
                                nc.tensor.matmul(pv, lt[:].bitcast(F32),
                                                 rhs.bitcast(F32))
                            else:
                                nc.tensor.matmul(pv, lt[:], rhs)
                            ev(ot[:, j: j + pk, :], pv)
                        j += pk
                    if t == 0:
                        rows = ot[0:VEP, 0, :]
                        dst = dst_dram[0: VEP * d].rearrange("(e i) -> e i",
                                                             e=VEP)
                        eng = nc.gpsimd if accum else nc.sync
                        eng.dma_start(dst, rows,
                                      **({"accum_op": ao.add} if accum
                                         else {}))
                    else:
                        if not accum:
                            dap = bass.AP(tensor=dst_dram,
                                          offset=VEP * t * d,
                                          ap=[[d, VEP], [VEP * d, g],
                                              [1, d]])
                            nc.sync.dma_start(dap, ot[WEP:128, :, :])
                        else:
                            j0 = 0
                            while j0 < g:
                                gg = min(8, g - j0)
                                dap = bass.AP(
                                    tensor=dst_dram,
                                    offset=(VEP * (t + j0)) * d,
                                    ap=[[d, VEP], [VEP * d, gg], [1, d]])
                                nc.gpsimd.dma_start(
                                    dap, ot[WEP:128, j0: j0 + gg, :],
                                    accum_op=ao.add)
                                j0 += gg

                if t_lo == 0:
                    do_group(0, 1)
                    yield
                t = max(1, t_lo)
                while t < t_hi:
                    g = min(G, t_hi - t)
                    do_group(t, g)
                    t += g
                    yield

            # per-stage h0/h1 tile cutoffs (each stage only reads what the
            # previous one has produced in its h0 pass)
            combs = sorted(COMB_DELAYS, key=lambda d: -_rev_tiles(d)[1])
            T0c = {d: HNP // (VEP * d) for d in combs}
            C1 = min(VEP * d * T0c[d] for d in combs)
            T0a0 = C1 // (VEP * AP_DELAYS[0])
            C2 = VEP * AP_DELAYS[0] * T0a0
            T0a1 = C2 // (VEP * AP_DELAYS[1])
            C3 = VEP * AP_DELAYS[1] * T0a1
            L0 = min((C3 // FH) // 32 * 32, 128)

            def drain(*gens, weights=None):
                gens = list(gens)
                ws = list(weights) if weights else [1] * len(gens)
                while gens:
                    nxt, nw = [], []
                    for g, w in zip(gens, ws):
                        alive = True
                        for _ in range(w):
                            try:
                                next(g)
                            except StopIteration:
                                alive = False
                                break
                        if alive:
                            nxt.append(g)
                            nw.append(w)
                    gens, ws = nxt, nw

            def gen_chain(*gens):
                for g in gens:
                    yield from g

            def gen_mix(h, p0, p1):
                for k in range(KC):
                    c0 = FH * h + FC * k
                    wetb = rvin.tile([128, FC], BF16, tag="rv_in")
                    mixt = chk.tile([128, FC], F32, tag="mtile")
                    nc.gpsimd.dma_start(wetb[p0:p1, :],
                                        w2vh[h][p0:p1, FC * k: FC * (k + 1)])
                    nc.vector.scalar_tensor_tensor(
                        mixt[p0:p1, :], wetb[p0:p1, :], t_wwet[p0:p1, 0:1],
                        yeq[p0:p1, c0: c0 + FC], op0=ao.mult, op1=ao.add)
                    nc.sync.dma_start(mdvh[h][p0:p1, FC * k: FC * (k + 1)],
                                      mixt[p0:p1, :].bitcast(F32R))
                    yield

            def gen_gap_zero():
                if _AP0_SIZE > _AP0_COVER:
                    gap = _AP0_SIZE - _AP0_COVER
                    ztg = rvout.tile([128, 2048], dt, tag="rv_out")
                    nc.gpsimd.memset(ztg[:].bitcast(F32), 0.0)
                    nc.sync.dma_start(
                        apdram[_AP0_COVER:].rearrange("(o f) -> o f", o=1),
                        ztg[0:1, 0:gap])
                yield

            # ---------------- pipeline ----------------
            drain(gen_compress(0))
            rev_all = gen_chain(
                *[epoch_filter(ydram, wetdram, d, t_lcT, t_lcT,
                               accum=(ci > 0), in_dt=BF16, t_hi=T0c[d])
                  for ci, d in enumerate(combs)],
                epoch_filter(wetdram, apdram, AP_DELAYS[0], t_laqT, t_laT,
                             False, G=8, in_dt=BF16, t_hi=T0a0),
                gen_gap_zero(),
                epoch_filter(apdram, wet2dram, AP_DELAYS[1], t_laqT, t_laT,
                             False, G=24, in_dt=BF16, t_hi=T0a1),
                gen_mix(0, 0, L0),
                *[epoch_filter(ydram, wetdram, d, t_lcT, t_lcT,
                               accum=(ci > 0), in_dt=BF16, t_lo=T0c[d])
                  for ci, d in enumerate(combs)],
                epoch_filter(wetdram, apdram, AP_DELAYS[0], t_laqT, t_laT,
                             False, G=8, in_dt=BF16, t_lo=T0a0),
                epoch_filter(apdram, wet2dram, AP_DELAYS[1], t_laqT, t_laT,
                             False, G=24, in_dt=BF16, t_lo=T0a1),
                gen_mix(0, L0, 128),
                gen_mix(1, 0, 128))
            drain(rev_all, gen_compress(1), weights=(1, 2))

            if with_collective:
                nc.gpsimd.collective_compute(
                    "ReduceScatter", ao.add,
                    replica_groups=[[0, 2, 4, 6], [1, 3, 5, 7]],
                    ins=[mixdram[0:NP].opt()],
                    outs=[rsdram.ap().opt()],
                )
            else:
                nc.sync.dma_start(rsdram[:], mixdram[0:CH])
            rs2 = rsdram[:].rearrange("(p f) -> p f", p=128)
            o2 = out[:].rearrange("(p f) -> p f", p=128)
            for hh in range(2):
                oc = rvout.tile([128, FC], dt, tag="rv_out")
                nc.sync.dma_start(oc[:], rs2[:, FC * hh: FC * (hh + 1)])
                nc.vector.tensor_scalar(oc[:], oc[:], float(-CEIL),
                                        float(CEIL), op0=ao.max, op1=ao.min)
                nc.sync.dma_start(o2[:, FC * hh: FC * (hh + 1)],
                                  oc[:].bitcast(F32))

    nc.compile()
    return nc


# ============================================================== host wrapper
_CACHE = {}

def _get_program():
    if "nc" not in _CACHE:
        _CACHE["nc"] = build_program()
    return _CACHE["nc"]


def _host_consts():
    if "consts" in _CACHE:
        return _CACHE["consts"]
    ident = np.eye(128, dtype=np.float32)
    Lc = np.ascontiguousarray((0.25 * _epoch_matrix_comb(FB)).T.astype(np.float32))
    La = np.ascontiguousarray(_epoch_matrix_ap(FB).T.astype(np.float32))
    Laq = np.ascontiguousarray(_epoch_matrix_ap(FB, quirk=True).T.astype(np.float32))
    relpow = np.ascontiguousarray(np.broadcast_to(
        (REL ** (np.arange(FC, dtype=np.float64) + 1.0)).astype(np.float32),
        (128, FC)))
    eqc = {}
    for t in range(4):
        T, Fm, SC = _track_eq_consts(t)
        eqc[t] = (np.ascontiguousarray(T.T.astype(np.float32)),
                  np.ascontiguousarray(Fm.T.astype(np.float32)),
                  np.ascontiguousarray(SC.astype(np.float32)))
    _CACHE["consts"] = (ident, Lc, La, Laq, relpow, eqc)
    return _CACHE["consts"]


def kernel(tracks, volumes, pans):
    tracks = np.ascontiguousarray(np.asarray(tracks, np.float32))
    volumes = np.asarray(volumes, np.float32)
    pans = np.asarray(pans, np.float32)

    angle = (pans.astype(np.float64) + 1.0) * 0.25 * math.pi
    lg, rg = np.cos(angle), np.sin(angle)
    ident, Lc, La, Laq, relpow, eqc = _host_consts()

    in_maps = []
    for core in range(8):
        t, ch = core // 2, core % 2
        xpad = np.zeros(NP, np.float32)
        xpad[:N] = tracks[t, ch]
        thT_np, ft_np, sc_np = eqc[t]
        w = float(volumes[t]) * float(lg[t] if ch == 0 else rg[t])
        has_rev = t >= 2
        w_dry = w * (1.0 - WET) if has_rev else w
        w_wet = w * WET if has_rev else 0.0
        in_maps.append({
            "x": xpad, "thT": thT_np, "ft": ft_np, "sc": sc_np,
            "ident": ident, "lcT": Lc, "laT": La, "laqT": Laq,
            "laTw": np.ascontiguousarray(La * np.float32(w_wet)),
            "laqTw": np.ascontiguousarray(Laq * np.float32(w_wet)),
            "relpow": relpow,
            "wdry": np.full((128, 1), w_dry, np.float32),
            "wwet": np.full((128, 1), w_wet, np.float32),
        })

    nc = _get_program()
    res = run_bass_kernel_spmd(nc, in_maps, list(range(8)))

    outp = np.zeros((2, N), np.float32)
    for ch in range(2):
        full = np.concatenate([res.results[2 * q + ch]["out"] for q in range(4)])
        outp[ch] = full[:N]
    return outp

